# revision 18
# baseline (speedup 1.0000x reference)
"""Causal self-attention (B=1, S=4096, D=768, H=12, dh=64) on 8 TRN2 NeuronCores.

Strategy (v3):
  - Sequence-parallel QKV projections + RoPE (each core projects 512 rows).
  - Queries are stride-8 interleaved (core c owns query rows c::8) so causal
    work balances and the program is SPMD-uniform; all per-core variation is
    input data (x slices, rope tables, masks).
  - KV ownership is by interleaved 128-row chunks: core c owns global chunks
    {8j+c}.  AllGather quarter u then delivers chunks 8u..8u+7 in causal
    order AND each rank's contribution is one whole chunk, so the gathered
    quarter reloads into SBUF as ONE DMA per tensor with >=768B runs.
  - K^T is gathered in fp8e4m3 (halves K gather bytes; QK runs as mixed
    fp8 x bf16 matmul), V in bf16 (fp8 V fails the error budget).  V carries
    a ones column per head (memset locally after reload, not gathered) that
    yields the softmax denominator through the AV matmul.
  - Every model input loads with a single strided DMA; K/V-path inputs load
    first so quarter 0 reaches the gather ASAP, Q-path and phase-C inputs
    are deferred behind quarter-0/1 gather traffic (the DMA engine pool is
    the phase-A bottleneck).
  - Attention: transposed layout (keys on partitions via S^T = K^T.T @ Q^T),
    3-chunk groups with a 48-column causal offset; exp runs on the scalar
    engine (one instruction per group x head).  The causal-band mask multiply
    runs on gpsimd, off the exp->AV critical path: group 0 is one full-range
    AV over masked exps carrying the single start=True (a matmul start resets
    the whole PSUM bank), later groups split into a mask-independent main
    region [xs+48, SL) and a 48-wide diagonal band emitted two groups late so
    the Pool latency hides.
  - PE emission order per group: QK(g) first (unblocks the next exp ASAP),
    then AVmain(g-1), then AVband(g-2), keeping the tensor engine dense so
    the p-state stays at max clock.
  - Softmax normalize: DVE reciprocal of the denominator row + gpsimd
    partition-broadcast + DVE multiply, overlapped with the next head-pair.
  - Output projection loops h-major so only the last two heads' matmuls wait
    on the final head-pair; output staged bf16 and widened on the host.
"""

import numpy as np
import ml_dtypes

import concourse.bass as bass
import concourse.bacc as bacc
import concourse.tile as tile
import concourse.mybir as mybir
import concourse.bass_utils as bass_utils

NCORES = 8
S = 4096
D = 768
H = 12
DH = 64
HALF = 32
P = 128
SL = S // NCORES          # 512 local queries / kv rows per core
KSUB = D // P             # 6
NKC = S // P              # 32 key chunks of 128
NQ = 4                    # pipelined AllGather quarters
VW = H * (DH + 1)         # 780: V row width incl. ones col per head
RQK = P * D               # per-rank K elems per quarter
RQV = P * VW              # per-rank V elems per quarter (incl. ones col)
NG3 = (NKC + 2) // 3      # 11 causal groups of <=3 chunks
F32 = mybir.dt.float32
BF16 = mybir.dt.bfloat16
F8 = mybir.dt.float8e4

_cache = {}


def _build(repeats=1, fake_gather=False, stop_after=None):
    nc = bacc.Bacc(
        "TRN2",
        target_bir_lowering=False,
        debug=False,
        enable_asserts=False,
        num_devices=1 if fake_gather else NCORES,
    )
    inp = {}
    for name, shape, dt in [
        ("xq", [D, SL], BF16),
        ("xkv", [D, SL], BF16),
        ("trig", [4, P, SL], BF16),   # cosq, sinq, cosk, sink
        ("mask3", [P, 3, 48], BF16),
        ("wq", [D, D], BF16),
        ("wk", [D, D], BF16),
        ("wv", [D, D], BF16),
        ("wo", [D, D], BF16),
    ]:
        inp[name] = nc.dram_tensor(name, shape, dt, kind="ExternalInput")
    out_d = nc.dram_tensor("out", [KSUB, P, SL], BF16, kind="ExternalOutput")

    with tile.TileContext(nc) as tc:
      for _rep in range(repeats):
        with (
            tc.tile_pool(name="persist", bufs=1) as persist,
            tc.tile_pool(name="dram", bufs=1, space="DRAM") as dram,
        ):
            # ---- persistent tiles ----
            qrot_t = [
                persist.tile([P, SL], BF16, name=f"qrot{s_}", tag=f"qrot{s_}")
                for s_ in range(KSUB)
            ]
            osb = persist.tile([P, KSUB, SL], BF16)  # head-pair i: head 2i on partitions 0-63, 2i+1 on 64-127
            mask_sb = persist.tile([P, 3, 48], BF16)
            trig_sb = persist.tile([P, 4, SL], BF16)
            wo_sb = persist.tile([P, KSUB, D], BF16)
            kvtK = [
                persist.tile([P, NCORES, D], F8, name=f"kvtK{u}", tag=f"kvtK{u}")
                for u in range(NQ)
            ]
            kvtV = [
                persist.tile(
                    [P, NCORES, H, DH + 1], BF16, name=f"kvtV{u}", tag=f"kvtV{u}"
                )
                for u in range(NQ)
            ]
            # K/V staging lives outside the phase-A pools: a pool close
            # inserts per-engine barriers gated on the staging-write DMAs,
            # which drain the DMA queue late and would stall phase-B work
            # queued behind the barrier.
            kvsK_t = [
                persist.tile([P, D], F8, name=f"kvsK{u}", tag=f"kvsK{u}")
                for u in range(NQ)
            ]
            kvsV_t = [
                persist.tile(
                    [P, H, DH + 1], BF16, name=f"kvsV{u}", tag=f"kvsV{u}"
                )
                for u in range(NQ)
            ]

            kvinK = dram.tile([NQ, RQK], F8)
            kvinV = dram.tile([NQ, RQV], BF16)
            kvoutK = [
                dram.tile(
                    [NCORES, RQK],
                    F8,
                    name=f"kvoutK{u}",
                    addr_space="Local" if fake_gather else "Shared",
                )
                for u in range(NQ)
            ]
            kvoutV = [
                dram.tile(
                    [NCORES, RQV],
                    BF16,
                    name=f"kvoutV{u}",
                    addr_space="Local" if fake_gather else "Shared",
                )
                for u in range(NQ)
            ]

            # ================= Phase A: projections + rope =================
            with (
                tc.tile_pool(name="pw", bufs=1) as pw,
                tc.tile_pool(name="px", bufs=1) as px,
                tc.tile_pool(name="pt", bufs=2) as pt,
                tc.tile_pool(name="psA", bufs=2, space="PSUM") as psA,
            ):
                w_sb = {}
                for name in ["wk", "wv", "wq"]:
                    w_sb[name] = pw.tile([P, KSUB, D], BF16, name=f"{name}_sb")
                xq_sb = px.tile([P, KSUB, SL], BF16)
                xkv_sb = px.tile([P, KSUB, SL], BF16)

                # PE warm-up: ~4.3us of dummy matmuls on a zeroed tile so
                # the p-state clock is at max when the K projection starts
                # (cold matmuls run at 1.2GHz until 3us of continuous work).
                pwarm = pt.tile([P, SL], BF16, name="pwarm", tag="pwarm")
                nc.vector.memset(pwarm[:], 0.0)
                for _wi in range(2):
                    pwp = psA.tile([P, SL], F32, name="paq", tag="paq")
                    for _wj in range(10):
                        nc.tensor.matmul(
                            pwp[:],
                            lhsT=pwarm[:, 0:P],
                            rhs=pwarm[:],
                            start=True,
                            stop=True,
                        )
                # K/V-path inputs first: the DMA engine pool is the phase-A
                # bottleneck and quarter 0 gates the attention start.
                nc.sync.dma_start(
                    w_sb["wk"][:],
                    inp["wk"].ap().rearrange("(ks p) m -> p ks m", p=P),
                )
                nc.sync.dma_start(
                    xkv_sb[:], inp["xkv"].ap().rearrange("(ks p) n -> p ks n", p=P)
                )
                nc.sync.dma_start(
                    trig_sb[:], inp["trig"].ap().rearrange("t p n -> p t n")
                )
                nc.sync.dma_start(
                    w_sb["wv"][:],
                    inp["wv"].ap().rearrange("(ks p) m -> p ks m", p=P),
                )
                cosq = trig_sb[:, 0, :]
                sinq = trig_sb[:, 1, :]
                cosk = trig_sb[:, 2, :]
                sink = trig_sb[:, 3, :]

                def load_q_inputs():
                    nc.sync.dma_start(
                        w_sb["wq"][:],
                        inp["wq"].ap().rearrange("(ks p) m -> p ks m", p=P),
                    )
                    nc.sync.dma_start(
                        xq_sb[:], inp["xq"].ap().rearrange("(ks p) n -> p ks n", p=P)
                    )

                def load_late_inputs():
                    nc.sync.dma_start(mask_sb[:], inp["mask3"].ap())
                    nc.sync.dma_start(
                        wo_sb[:], inp["wo"].ap().rearrange("(i p) e -> p i e", p=P)
                    )

                def project_rope_q(s):
                    paq = psA.tile([P, SL], F32, name="paq", tag="paq")
                    for ks in range(KSUB):
                        nc.tensor.matmul(
                            paq[:],
                            lhsT=w_sb["wq"][:, ks, s * P : (s + 1) * P],
                            rhs=xq_sb[:, ks, :],
                            start=(ks == 0),
                            stop=(ks == KSUB - 1),
                        )
                    pab = pt.tile([P, SL], BF16, name="pabq", tag="pabq")
                    nc.scalar.copy(pab[:], paq[:])
                    swp = pt.tile([P, SL], BF16, name="swpq", tag="swpq")
                    for (dd, ss2) in [(0, 32), (32, 0), (64, 96), (96, 64)]:
                        nc.vector.tensor_copy(
                            swp[dd : dd + 32, :], pab[ss2 : ss2 + 32, :]
                        )
                    t1 = pt.tile([P, SL], BF16, name="t1q", tag="t1q")
                    t2 = pt.tile([P, SL], BF16, name="t2q", tag="t2q")
                    nc.vector.tensor_mul(t1[:], pab[:], cosq)
                    nc.vector.tensor_mul(t2[:], swp[:], sinq)
                    nc.vector.tensor_add(qrot_t[s][:], t1[:], t2[:])

                for u in range(NQ):
                    kvsK = kvsK_t[u]
                    kvsV = kvsV_t[u]
                    nc.vector.memset(kvsV[:, :, DH : DH + 1], 1.0)
                    for s in range(KSUB):
                        pak = psA.tile([P, P], F32, name="pak", tag="pak")
                        for ks in range(KSUB):
                            nc.tensor.matmul(
                                pak[:],
                                lhsT=w_sb["wk"][:, ks, s * P : (s + 1) * P],
                                rhs=xkv_sb[:, ks, u * P : (u + 1) * P],
                                start=(ks == 0),
                                stop=(ks == KSUB - 1),
                            )
                        pab = pt.tile([P, P], BF16, name="pabk", tag="pabk")
                        nc.scalar.copy(pab[:], pak[:])
                        swp = pt.tile([P, P], BF16, name="swpk", tag="swpk")
                        for (dd, ss2) in [(0, 32), (32, 0), (64, 96), (96, 64)]:
                            nc.vector.tensor_copy(
                                swp[dd : dd + 32, :], pab[ss2 : ss2 + 32, :]
                            )
                        t1 = pt.tile([P, P], BF16, name="t1k", tag="t1k")
                        t2 = pt.tile([P, P], BF16, name="t2k", tag="t2k")
                        nc.vector.tensor_mul(
                            t1[:], pab[:], cosk[:, u * P : (u + 1) * P]
                        )
                        nc.vector.tensor_mul(
                            t2[:], swp[:], sink[:, u * P : (u + 1) * P]
                        )
                        nc.vector.tensor_add(
                            kvsK[:, s * P : (s + 1) * P], t1[:], t2[:]
                        )
                    # j slices are [P, SL] so each stays inside one 2KB PSUM
                    # bank (matmul outputs may not straddle banks)
                    pv = psA.tile([P, 2, SL], F32, name="pv", tag="pv")
                    for j in range(2):
                        for ks in range(KSUB):
                            nc.tensor.matmul(
                                pv[:, j, 0 : D // 2],
                                lhsT=xkv_sb[:, ks, u * P : (u + 1) * P],
                                rhs=w_sb["wv"][:, ks, j * (D // 2) : (j + 1) * (D // 2)],
                                start=(ks == 0),
                                stop=(ks == KSUB - 1),
                            )
                    for j in range(2):
                        nc.scalar.copy(
                            kvsV[:, j * 6 : (j + 1) * 6, 0:DH],
                            pv[:, j, 0 : D // 2].rearrange("p (h d) -> p h d", d=DH),
                        )
                    nc.sync.dma_start(
                        kvinK[u].rearrange("(p x) -> p x", p=P), kvsK[:]
                    )
                    nc.sync.dma_start(
                        kvinV[u].rearrange("(p h d) -> p h d", p=P, h=H), kvsV[:]
                    )
                    if fake_gather:
                        for c in range(NCORES):
                            nc.sync.dma_start(kvoutK[u][c], kvinK[u])
                            nc.sync.dma_start(kvoutV[u][c], kvinV[u])
                    else:
                        nc.gpsimd.collective_compute(
                            "AllGather",
                            mybir.AluOpType.bypass,
                            replica_groups=[list(range(NCORES))],
                            ins=[kvinK[u].opt()],
                            outs=[kvoutK[u][:].opt()],
                        )
                        nc.gpsimd.collective_compute(
                            "AllGather",
                            mybir.AluOpType.bypass,
                            replica_groups=[list(range(NCORES))],
                            ins=[kvinV[u].opt()],
                            outs=[kvoutV[u][:].opt()],
                        )
                    nc.sync.dma_start(
                        kvtK[u][:],
                        kvoutK[u][:].rearrange("c (p x) -> p c x", p=P),
                    )
                    nc.sync.dma_start(
                        kvtV[u][:],
                        kvoutV[u][:].rearrange("c (p h d) -> p c h d", p=P, h=H),
                    )
                    if u == 0:
                        load_q_inputs()
                        project_rope_q(0)
                        project_rope_q(1)
                    elif u == 1:
                        load_late_inputs()
                        project_rope_q(2)
                        project_rope_q(3)
                    elif u == 2:
                        project_rope_q(4)
                        project_rope_q(5)

            # ================= Phase B: attention =================
            # Quarter-aligned waves: wave w covers causal groups whose chunks
            # live in quarters <= w, iterating ALL head-pairs per wave, so
            # attention consumption matches the gather pipeline's delivery
            # rate instead of head-pair 0 burning through every quarter and
            # stalling.  Per-(hp,wave) AV partials accumulate in PSUM and are
            # flushed into an SBUF accumulator by DVE adds.
            if stop_after == "A":
                continue
            WAVES = [[0, 1], [2, 3, 4], [5, 6, 7], [8, 9, 10]]
            with (
                tc.tile_pool(name="pe", bufs=3) as pe,
                tc.tile_pool(name="pn", bufs=2) as pn,
                tc.tile_pool(name="pacc", bufs=1) as pacc,
                tc.tile_pool(name="psS", bufs=1, space="PSUM") as psS,
                tc.tile_pool(name="psO", bufs=1, space="PSUM") as psO,
            ):
                acc = pacc.tile([DH + 1, H, SL], F32)
                for w, groups in enumerate(WAVES):
                  g_min = groups[0]
                  xsw = 48 * g_min
                  for hp in range(H // 2):
                    s = hp
                    ots = [
                        psO.tile([DH + 1, SL], F32, name=f"ot{j}", tag=f"ot{j}")
                        for j in range(2)
                    ]

                    stash = {}

                    # The wave's first group emits one full-range AV over
                    # masked exps carrying the single start=True (a matmul
                    # start resets the whole PSUM bank, so the first-executed
                    # AV must cover the wave's whole column range); later
                    # groups split into a mask-free main region and a 48-wide
                    # diagonal band emitted late so the Pool mask hides.
                    def emit_avmain(g):
                        xs = xsw if g == g_min else 48 * g + 48
                        if xs >= SL:
                            return
                        expss = stash[g]
                        for j in range(2):
                            h = 2 * hp + j
                            for i, kc in enumerate(range(3 * g, min(3 * g + 3, NKC))):
                                nc.tensor.matmul(
                                    ots[j][:, xs:SL],
                                    lhsT=kvtV[kc // 8][:, kc % 8, h, :],
                                    rhs=expss[j][:, i, xs:SL],
                                    start=(g == g_min and i == 0),
                                    stop=False,
                                    skip_group_check=True,
                                )

                    def emit_avband(g, last=False):
                        xs = 48 * g
                        mw = min(48, SL - xs)
                        expss = stash.pop(g)
                        chunks = list(range(3 * g, min(3 * g + 3, NKC)))
                        for j in range(2):
                            h = 2 * hp + j
                            for i, kc in enumerate(chunks):
                                if g == g_min:
                                    continue  # covered by the full-range main
                                nc.tensor.matmul(
                                    ots[j][:, xs : xs + mw],
                                    lhsT=kvtV[kc // 8][:, kc % 8, h, :],
                                    rhs=expss[j][:, i, xs : xs + mw],
                                    start=False,
                                    stop=(last and i == len(chunks) - 1),
                                    skip_group_check=True,
                                )

                    for gi, g in enumerate(groups):
                        chunks = list(range(3 * g, min(3 * g + 3, NKC)))
                        nch = len(chunks)
                        xs = 48 * g
                        mw = min(48, SL - xs)
                        sts = [
                            psS.tile([P, 3, SL], F32, name=f"st{j}", tag=f"st{j}")
                            for j in range(2)
                        ]
                        for j in range(2):
                            off = 64 * j
                            for i, kc in enumerate(chunks):
                                nc.tensor.matmul(
                                    sts[j][:, i, xs:SL],
                                    lhsT=kvtK[kc // 8][
                                        off : off + 64, kc % 8, s * P : (s + 1) * P
                                    ],
                                    rhs=qrot_t[s][off : off + 64, xs:SL],
                                    start=True,
                                    stop=True,
                                )
                        expss = []
                        for j in range(2):
                            exps = pe.tile(
                                [P, 3, SL], BF16, name=f"exps{j}", tag=f"exps{j}"
                            )
                            nc.scalar.activation(
                                exps[:, 0:nch, xs:SL],
                                sts[j][:, 0:nch, xs:SL],
                                mybir.ActivationFunctionType.Exp,
                                scale=0.125,
                            )
                            nc.vector.tensor_mul(
                                exps[:, 0:nch, xs : xs + mw],
                                exps[:, 0:nch, xs : xs + mw],
                                mask_sb[:, 0:nch, 0:mw],
                            )
                            expss.append(exps)
                        stash[g] = expss
                        if gi >= 1:
                            emit_avmain(groups[gi - 1])
                        if gi >= 2:
                            emit_avband(groups[gi - 2])
                    emit_avmain(groups[-1])
                    if len(groups) >= 2:
                        emit_avband(groups[-2])
                    emit_avband(groups[-1], last=True)

                    # flush the wave's PSUM partial into the SBUF accumulator
                    for j in range(2):
                        h = 2 * hp + j
                        if w == 0:
                            nc.vector.tensor_copy(acc[:, h, :], ots[j][:])
                        else:
                            nc.vector.tensor_add(
                                acc[:, h, xsw:SL],
                                acc[:, h, xsw:SL],
                                ots[j][:, xsw:SL],
                            )
                        if w == len(WAVES) - 1:
                            den = pn.tile([1, SL], F32, name="den", tag="den")
                            nc.vector.tensor_copy(den[0:1, :], acc[64:65, h, :])
                            recip = pn.tile([1, SL], F32, name="recip", tag="recip")
                            nc.vector.reciprocal(recip[:], den[:])
                            recipb = pn.tile(
                                [64, SL], F32, name="recipb", tag="recipb"
                            )
                            nc.gpsimd.partition_broadcast(recipb[:], recip[:])
                            nc.vector.tensor_mul(
                                osb[64 * j : 64 * j + 64, hp, :],
                                acc[0:64, h, :],
                                recipb[:],
                            )

            # ================= Phase C: output projection =================
            if stop_after == "B":
                continue
            with (
                tc.tile_pool(name="pco", bufs=2) as pco,
                tc.tile_pool(name="psC", bufs=2, space="PSUM") as psC,
            ):
                for m in range(KSUB):
                    outp = psC.tile([P, SL], F32, name="outp", tag="outp")
                    for i in range(KSUB):
                        nc.tensor.matmul(
                            outp[:],
                            lhsT=wo_sb[:, i, m * P : (m + 1) * P],
                            rhs=osb[:, i, :],
                            start=(i == 0),
                            stop=(i == KSUB - 1),
                        )
                    ob = pco.tile([P, SL], BF16, name="ob", tag="ob")
                    nc.vector.tensor_copy(ob[:], outp[:])
                    nc.sync.dma_start(out_d.ap()[m], ob[:])

    nc.compile()
    return nc


def _host_prep(x, position_ids, Wq, Wk, Wv, Wo):
    x2 = np.asarray(x, dtype=np.float32).reshape(S, D)
    pos = np.asarray(position_ids).reshape(S)

    fraction = (2.0 * np.arange(HALF, dtype=np.float32) / DH).astype(np.float32)
    timescale = (10000.0 ** fraction).astype(np.float32)  # [32]

    def tables(p_vec):
        sinu = (p_vec[None, :].astype(np.float32) / timescale[:, None]).astype(
            np.float32
        )
        cos = np.tile(np.cos(sinu).astype(np.float32), (4, 1))
        sin = np.sin(sinu).astype(np.float32)
        # signed for the swap formulation: first-half rows get -sin (they
        # subtract the swapped second half), second-half rows get +sin.
        sin = np.concatenate([-sin, sin, -sin, sin], axis=0)
        return cos, sin

    bf = ml_dtypes.bfloat16
    weights = {
        "wq": np.ascontiguousarray(np.asarray(Wq, dtype=np.float32)).astype(bf),
        "wk": np.ascontiguousarray(np.asarray(Wk, dtype=np.float32)).astype(bf),
        "wv": np.ascontiguousarray(np.asarray(Wv, dtype=np.float32)).astype(bf),
        "wo": np.ascontiguousarray(np.asarray(Wo, dtype=np.float32)).astype(bf),
    }

    in_maps = []
    for c in range(NCORES):
        qrows = np.arange(SL) * NCORES + c
        # kv rows: core c owns global 128-chunks {8j+c}
        kvrows = (
            (np.arange(NQ) * NCORES + c)[:, None] * P + np.arange(P)[None, :]
        ).ravel()
        cosq, sinq = tables(pos[qrows])
        cosk, sink = tables(pos[kvrows])
        trig = np.stack([cosq, sinq, cosk, sink], axis=0).astype(bf)
        pp = np.arange(P)[:, None, None]
        ii = np.arange(3)[None, :, None]
        jj = np.arange(48)[None, None, :]
        mask3 = (P * ii + pp <= NCORES * jj + c).astype(bf)
        m = {
            "xq": np.ascontiguousarray(x2[qrows, :].T).astype(bf),
            "xkv": np.ascontiguousarray(x2[kvrows, :].T).astype(bf),
            "trig": trig,
            "mask3": mask3,
        }
        m.update(weights)
        in_maps.append(m)
    return in_maps


def kernel(x, position_ids, Wq, Wk, Wv, Wo):
    if "nc" not in _cache:
        _cache["nc"] = _build()
    nc = _cache["nc"]
    in_maps = _host_prep(x, position_ids, Wq, Wk, Wv, Wo)
    res = bass_utils.run_bass_kernel_spmd(
        nc, in_maps, core_ids=list(range(NCORES))
    )
    out = np.empty((1, S, D), dtype=np.float32)
    for c in range(NCORES):
        outT = res.results[c]["out"].astype(np.float32).reshape(D, SL)
        out[0, c::NCORES, :] = outT.T
    return out
